# revision 17
# baseline (speedup 1.0000x reference)
"""Trainium2 Bass kernel for nn_Block_2010044694563 (dense transformer block).

B=4, S=2048, D=768, H=12 heads of 64. 8 NeuronCores, no collectives:
core c handles batch c//2, query-half c%2. Each core receives its batch's
2048 tokens rolled so its 1024 query rows come first, computes LN1 + K/V
over all 2048 local tokens (the only redundant work), attention for its
1024 queries x 12 heads, then out-proj + FFN on its 1024 rows.

Precision: fp32 storage / elementwise / PSUM accumulation. QKV/out-proj
matmuls run in fp8e4 DoubleRow perf mode (2 contraction rows per
partition, half-rate rows): weights are pre-scaled x32 on the host so
their 0.02-std values clear the fp8 subnormal cliff, and the 1/32 (and
attention's 1/64 fp8 range scale) are folded into the PSUM-evacuation
scalar ops. Scores stay bf16 (64-deep contraction can't use DoubleRow);
softmax exp emits fp8 directly from the ACT engine and PV consumes it
with V stationary in DoubleRow over key-chunk pairs. The attention mask
is folded into V and the denominator ones-column (zeroed keys drop out
of both numerator and denominator), so exp needs no per-key bias and
LN stats run on the vector engine to keep ACT free for exp.
FFN stays bf16 for the error budget.
"""

import numpy as np
import ml_dtypes

B, S, D, H = 4, 2048, 768, 12
HS = D // H           # 64
P = 128
NT = S                # local tokens per core (whole batch)
NQ = S // 2           # query tokens per core
TCH = NT // P         # 16 token chunks
QCH = NQ // P         # 8 query chunks
KC = D // P           # 6 feature chunks
EPS = 1e-5
SCALE = float(D) ** -0.5
VW = 96            # V columns padded to a 32-multiple for DoubleRow ldweights
BF16 = ml_dtypes.bfloat16
FP8 = ml_dtypes.float8_e4m3
WS = 32.0             # host-side fp8 weight scale
AS = 64.0             # attention-probs fp8 range scale

_PROGRAM_CACHE = {}


def _build_program(gelu=True):
    import concourse.bass as bass
    import concourse.mybir as mybir
    import concourse.tile as tile
    from concourse import bacc
    from concourse.masks import make_identity
    from contextlib import ExitStack

    f32 = mybir.dt.float32
    bf16 = mybir.dt.bfloat16
    fp8 = mybir.dt.float8e4
    AF = mybir.ActivationFunctionType
    OP = mybir.AluOpType
    DR = mybir.MatmulPerfMode.DoubleRow

    nc = bacc.Bacc(None, target_bir_lowering=False)

    x_d = nc.dram_tensor("x_local", [NT, D], f32, kind="ExternalInput")
    mv_d = nc.dram_tensor("maskv", [NT], f32, kind="ExternalInput")
    mr_d = nc.dram_tensor("maskrep", [P, TCH, H, VW], fp8, kind="ExternalInput")
    wq_d = nc.dram_tensor("wq", [D, D], fp8, kind="ExternalInput")
    wk_d = nc.dram_tensor("wk", [D, D], fp8, kind="ExternalInput")
    wv_d = nc.dram_tensor("wv", [D, D], fp8, kind="ExternalInput")
    wo_d = nc.dram_tensor("wo", [D, D], fp8, kind="ExternalInput")
    w1_d = nc.dram_tensor("w1", [D, D], bf16, kind="ExternalInput")
    w2_d = nc.dram_tensor("w2", [D, D], bf16, kind="ExternalInput")
    bq_d = nc.dram_tensor("bq", [D], f32, kind="ExternalInput")
    bk_d = nc.dram_tensor("bk", [D], f32, kind="ExternalInput")
    bo_d = nc.dram_tensor("bo2", [D], f32, kind="ExternalInput")
    b1_d = nc.dram_tensor("b1f", [D], f32, kind="ExternalInput")
    b2_d = nc.dram_tensor("b2f", [D], f32, kind="ExternalInput")
    out_d = nc.dram_tensor("out", [NQ, D], f32, kind="ExternalOutput")

    with tile.TileContext(nc) as tc, ExitStack() as ctx:
        const = ctx.enter_context(tc.tile_pool(name="const", bufs=1))
        glob = ctx.enter_context(tc.tile_pool(name="glob", bufs=1))
        rot = ctx.enter_context(tc.tile_pool(name="rot", bufs=1))
        wpool = ctx.enter_context(tc.tile_pool(name="wpool", bufs=1))

        # ---- constants ----
        ident = const.tile([P, P], bf16)
        make_identity(nc, ident)
        mv_sb = const.tile([P, TCH], f32)
        nc.sync.dma_start(out=mv_sb, in_=mv_d[:].rearrange("(c p) -> p c", p=P))
        bq_sb = const.tile([P, KC], f32)
        nc.sync.dma_start(out=bq_sb, in_=bq_d[:].rearrange("(c p) -> p c", p=P))
        bk_sb = const.tile([P, KC], f32)
        nc.sync.dma_start(out=bk_sb, in_=bk_d[:].rearrange("(c p) -> p c", p=P))
        b1_sb = const.tile([P, KC], f32)
        nc.sync.dma_start(out=b1_sb, in_=b1_d[:].rearrange("(c p) -> p c", p=P))
        # per-feature biases broadcast across partitions (token-major use)
        bo_b = const.tile([P, D], f32)
        _bo = bo_d[:]
        nc.gpsimd.dma_start(
            out=bo_b, in_=bass.AP(tensor=_bo.tensor, offset=_bo.offset, ap=[[0, P], _bo.ap[0]])
        )
        b2_b = const.tile([P, D], f32)
        _b2 = b2_d[:]
        nc.gpsimd.dma_start(
            out=b2_b, in_=bass.AP(tensor=_b2.tensor, offset=_b2.offset, ap=[[0, P], _b2.ap[0]])
        )

        # whole-kernel persistent: attention output (feature-major, normalized)
        oT = glob.tile([P, KC, NQ], fp8)
        xq = glob.tile([P, QCH, D], f32)

        x_r = x_d[:].rearrange("(c p) d -> c p d", p=P)

        # phase-scoped pools (stack order: apool outlives hpool)
        apool = tc.alloc_tile_pool(name="apool", bufs=1)
        hpool = tc.alloc_tile_pool(name="hpool", bufs=1)
        ps_a = tc.alloc_tile_pool(name="ps_a", bufs=1, space="PSUM")

        qT = apool.tile([P, KC, NQ], bf16)       # Q^T (head-pair-major)
        kT = apool.tile([P, KC, NT], bf16)       # K^T
        vA = apool.tile([P, TCH, H, VW], fp8)  # V/32 per (tok chunk, head): [V | mask | 0pad]
        hT = hpool.tile([P, KC, NT], fp8)        # LN1(x)^T, feature-major

        # full-width image of vA: zeros where V lands (overwritten by the
        # projection evacs), mask in the denominator column, zero pad to the
        # 32-multiple stationary width DoubleRow needs. One contiguous DMA.
        nc.sync.dma_start(out=vA[:, :, :, :], in_=mr_d[:, :, :, :])

        # ================= Phase 1: LN1 + transpose to h^T =================
        with nc.named_scope("ln1"):
            for t in range(TCH):
                xt = rot.tile([P, D], f32, tag="xin", bufs=3, name=f"xt{t}")
                nc.sync.dma_start(out=xt, in_=x_r[t])
                scr = rot.tile([P, D], bf16, tag="xn", bufs=4, name=f"scr{t}")
                ssq = rot.tile([P, 1], f32, tag="ssq", bufs=4, name=f"ssq{t}")
                msum = rot.tile([P, 1], f32, tag="msum", bufs=4, name=f"msum{t}")
                nc.scalar.activation(scr, xt, AF.Square, accum_out=ssq)
                nc.vector.reduce_sum(out=msum, in_=xt, axis=mybir.AxisListType.X)
                # var = ssq/D - (msum/D)^2 ; rstd = sqrt(1/(var+eps))
                mu = rot.tile([P, 1], f32, tag="mu", bufs=4, name=f"mu{t}")
                nc.vector.tensor_scalar_mul(out=mu, in0=msum, scalar1=1.0 / D)
                mu2 = rot.tile([P, 1], f32, tag="mu2", bufs=4, name=f"mu2{t}")
                nc.vector.tensor_tensor(mu2, mu, mu, OP.mult)
                ve = rot.tile([P, 1], f32, tag="ve", bufs=4, name=f"ve_{t}")
                nc.vector.tensor_scalar(
                    out=ve, in0=ssq, scalar1=1.0 / D, scalar2=EPS,
                    op0=OP.mult, op1=OP.add,
                )
                nc.vector.tensor_tensor(ve, ve, mu2, OP.subtract)
                rstd = rot.tile([P, 1], f32, tag="rstd", bufs=4, name=f"rstd{t}")
                nc.vector.reciprocal_approx_fast(out=rstd, in_=ve)
                nc.scalar.activation(rstd, rstd, AF.Sqrt, scale=1.0)
                nmr = rot.tile([P, 1], f32, tag="nmr", bufs=4, name=f"nmr{t}")
                nc.vector.tensor_tensor(nmr, mu, rstd, OP.mult)
                nc.vector.tensor_scalar_mul(out=nmr, in0=nmr, scalar1=-1.0)
                xn = rot.tile([P, D], bf16, tag="xn", bufs=4, name=f"xn{t}")
                nc.vector.tensor_scalar(
                    out=xn, in0=xt, scalar1=rstd, scalar2=nmr,
                    op0=OP.mult, op1=OP.add,
                )
                pt = ps_a.tile([P, KC, P], bf16, tag="tp", bufs=3, name=f"pt{t}")
                for f in range(KC):
                    nc.tensor.transpose(pt[:, f], xn[:, f * P : (f + 1) * P], ident)
                nc.vector.tensor_copy(out=hT[:, :, t * P : (t + 1) * P], in_=pt)

        # residual rows for the out-projection: queued after LN1's x loads
        for t in range(QCH):
            nc.sync.dma_start(out=xq[:, t], in_=x_r[t])
            nc.vector.tensor_tensor(xq[:, t], xq[:, t], bo_b, OP.add)

        # ================= Phase 2: Q/K/V projections (fp8 DoubleRow) ======
        with nc.named_scope("qkv"):
            wv_sb = wpool.tile([P, KC, D], fp8, tag="w8", bufs=3, name="wv_sb")
            nc.sync.dma_start(out=wv_sb, in_=wv_d[:].rearrange("(c p) n -> p c n", p=P))
            wq_sb = wpool.tile([P, KC, D], fp8, tag="w8", bufs=3, name="wq_sb")
            nc.sync.dma_start(out=wq_sb, in_=wq_d[:].rearrange("(c p) n -> p c n", p=P))
            wk_sb = wpool.tile([P, KC, D], fp8, tag="w8", bufs=3, name="wk_sb")
            nc.sync.dma_start(out=wk_sb, in_=wk_d[:].rearrange("(c p) n -> p c n", p=P))
            for t in range(TCH):
                for n2 in range(2):
                    ps = ps_a.tile([P, 384], f32, tag="mm", bufs=4, name=f"psv{t}_{n2}")
                    for c in range(KC // 2):
                        nc.tensor.matmul(
                            ps,
                            lhsT=hT[:, 2 * c : 2 * c + 2, t * P : (t + 1) * P],
                            rhs=wv_sb[:, 2 * c : 2 * c + 2, n2 * 384 : (n2 + 1) * 384],
                            start=(c == 0), stop=(c == KC // 2 - 1),
                            perf_mode=DR,
                        )
                    # x(1/32) de-scales the fp8 weights; mask zeroes dead keys
                    nc.vector.tensor_scalar(
                        out=vA[:, t, n2 * 6 : (n2 + 1) * 6, 0:HS],
                        in0=ps.rearrange("p (h d) -> p h d", h=6),
                        scalar1=mv_sb[:, t : t + 1],
                        scalar2=None,
                        op0=OP.mult,
                    )
            for hp in range(KC):
                for n in range(NQ // 512):
                    psq = ps_a.tile([P, 512], f32, tag="mm", bufs=4, name=f"psq{hp}_{n}")
                    for c in range(KC // 2):
                        nc.tensor.matmul(
                            psq,
                            lhsT=wq_sb[:, 2 * c : 2 * c + 2, hp * P : (hp + 1) * P],
                            rhs=hT[:, 2 * c : 2 * c + 2, n * 512 : (n + 1) * 512],
                            start=(c == 0), stop=(c == KC // 2 - 1),
                            perf_mode=DR,
                        )
                    nc.vector.tensor_scalar(
                        out=qT[:, hp, n * 512 : (n + 1) * 512], in0=psq,
                        scalar1=1.0 / WS, scalar2=bq_sb[:, hp : hp + 1],
                        op0=OP.mult, op1=OP.add,
                    )
                for n in range(NT // 512):
                    psk = ps_a.tile([P, 512], f32, tag="mm", bufs=4, name=f"psk{hp}_{n}")
                    for c in range(KC // 2):
                        nc.tensor.matmul(
                            psk,
                            lhsT=wk_sb[:, 2 * c : 2 * c + 2, hp * P : (hp + 1) * P],
                            rhs=hT[:, 2 * c : 2 * c + 2, n * 512 : (n + 1) * 512],
                            start=(c == 0), stop=(c == KC // 2 - 1),
                            perf_mode=DR,
                        )
                    nc.vector.tensor_scalar(
                        out=kT[:, hp, n * 512 : (n + 1) * 512], in0=psk,
                        scalar1=1.0 / WS, scalar2=bk_sb[:, hp : hp + 1],
                        op0=OP.mult, op1=OP.add,
                    )
        ps_a.release()
        hpool.release()

        # ================= Phase 3: attention ==============================
        # scores^T[k,q] per head (bf16, contraction=64), exp fused with the
        # D^-0.5 scale straight to fp8, PV with V stationary in DoubleRow over
        # key-chunk pairs; the mask column of V gives denominators for free.
        wo_sb = wpool.tile([P, KC, D], fp8, tag="w8", bufs=3, name="wo_sb")
        nc.sync.dma_start(out=wo_sb, in_=wo_d[:].rearrange("(c p) n -> p c n", p=P))
        ps_b = tc.alloc_tile_pool(name="ps_b", bufs=1, space="PSUM")
        dpool = tc.alloc_tile_pool(name="dpool", bufs=1, space="DRAM")

        def emit_norm(pv, h, qc):
            # deferred softmax-normalize: AS/denom (fast recip via SBUF copy),
            # replicate across partitions through a DRAM-roundtrip broadcast
            # DMA, then scale O rows during the PSUM evacuation.
            qs = slice(qc * 512, (qc + 1) * 512)
            hr = slice((h % 2) * HS, (h % 2) * HS + HS)
            pvr = rot.tile([1, 512], f32, tag="pvr", bufs=2, name=f"pvr{h}_{qc}")
            nc.vector.tensor_scalar_mul(out=pvr, in0=pv[HS : HS + 1, :], scalar1=1.0 / AS)
            rsb = rot.tile([1, 512], f32, tag="rsb", bufs=2, name=f"rsb{h}_{qc}")
            nc.vector.reciprocal_approx_fast(out=rsb, in_=pvr)
            rd = dpool.tile([1, 512], f32, tag="rd", bufs=2, name=f"rd{h}_{qc}")
            nc.sync.dma_start(out=rd, in_=rsb)
            rrs = rot.tile([HS, 512], f32, tag="rrs", bufs=2, name=f"rrs{h}_{qc}")
            nc.gpsimd.dma_start(
                out=rrs,
                in_=bass.AP(
                    tensor=rd.tensor, offset=rd.offset,
                    ap=[[0, HS]] + [list(a) for a in rd.ap[1:]],
                ),
            )
            nc.vector.tensor_tensor(oT[hr, h // 2, qs], pv[0:HS, :], rrs, OP.mult)

        with nc.named_scope("attn"):
            pending = None
            for h in range(H):
                hd = slice((h % 2) * HS, (h % 2) * HS + HS)
                for qc in range(NQ // 512):
                    qs = slice(qc * 512, (qc + 1) * 512)
                    pv = ps_b.tile([VW, 512], f32, tag="pv", bufs=3, name=f"pv{h}_{qc}")
                    for jp in range(TCH // 2):
                        sc = ps_b.tile([P, 2, 512], f32, tag="sc", bufs=2, name=f"sc{h}_{qc}_{jp}")
                        for jj in range(2):
                            js = slice((2 * jp + jj) * P, (2 * jp + jj + 1) * P)
                            nc.tensor.matmul(
                                sc[:, jj, :], lhsT=kT[hd, h // 2, js], rhs=qT[hd, h // 2, qs],
                                start=True, stop=True,
                            )
                        ex = rot.tile([P, 2, 512], fp8, tag="expT", bufs=3, name=f"ex{h}_{qc}_{jp}")
                        nc.scalar.activation(ex, sc, AF.Exp, scale=SCALE)
                        nc.tensor.matmul(
                            pv,
                            lhsT=vA[:, 2 * jp : 2 * jp + 2, h, :],
                            rhs=ex,
                            start=(jp == 0), stop=(jp == TCH // 2 - 1),
                            perf_mode=DR,
                        )
                    if pending is not None:
                        emit_norm(*pending)
                    pending = (pv, h, qc)
            emit_norm(*pending)
        apool.release()
        ps_b.release()
        dpool.release()

        # ================= Phase 4: out-projection + residual ==============
        lpool = tc.alloc_tile_pool(name="lpool", bufs=1)
        ps_c = tc.alloc_tile_pool(name="ps_c", bufs=1, space="PSUM")
        x2 = lpool.tile([P, QCH, D], f32)
        h2T = lpool.tile([P, KC, NQ], bf16)
        gT = lpool.tile([P, KC, NQ], bf16)
        with nc.named_scope("proj"):
            for qm in range(QCH):
                for n2 in range(2):
                    ns = slice(n2 * 384, (n2 + 1) * 384)
                    ps = ps_c.tile([P, 384], f32, tag="mm", bufs=4, name=f"pso{qm}_{n2}")
                    for c in range(KC // 2):
                        nc.tensor.matmul(
                            ps,
                            lhsT=oT[:, 2 * c : 2 * c + 2, qm * P : (qm + 1) * P],
                            rhs=wo_sb[:, 2 * c : 2 * c + 2, ns],
                            start=(c == 0), stop=(c == KC // 2 - 1),
                            perf_mode=DR,
                        )
                    # 1/(AS*WS) undoes the attn fp8 range and weight scales
                    osc = rot.tile([P, 384], bf16, tag="osc", bufs=3, name=f"osc{qm}_{n2}")
                    nc.vector.tensor_scalar_mul(out=osc, in0=ps, scalar1=1.0 / (AS * WS))
                    nc.vector.tensor_tensor(x2[:, qm, ns], osc, xq[:, qm, ns], OP.add)

        # ================= Phase 5: LN2 + transpose =================
        with nc.named_scope("ln2"):
            for t in range(QCH):
                scr = rot.tile([P, D], bf16, tag="xn", bufs=4, name=f"scr2_{t}")
                ssq = rot.tile([P, 1], f32, tag="ssq", bufs=4, name=f"ssq2_{t}")
                msum = rot.tile([P, 1], f32, tag="msum", bufs=4, name=f"msum2_{t}")
                nc.scalar.activation(scr, x2[:, t], AF.Square, accum_out=ssq)
                nc.vector.reduce_sum(out=msum, in_=x2[:, t], axis=mybir.AxisListType.X)
                mu = rot.tile([P, 1], f32, tag="mu", bufs=4, name=f"mu_2{t}")
                nc.vector.tensor_scalar_mul(out=mu, in0=msum, scalar1=1.0 / D)
                mu2 = rot.tile([P, 1], f32, tag="mu2", bufs=4, name=f"mu2_2{t}")
                nc.vector.tensor_tensor(mu2, mu, mu, OP.mult)
                ve = rot.tile([P, 1], f32, tag="ve", bufs=4, name=f"ve2_{t}")
                nc.vector.tensor_scalar(
                    out=ve, in0=ssq, scalar1=1.0 / D, scalar2=EPS,
                    op0=OP.mult, op1=OP.add,
                )
                nc.vector.tensor_tensor(ve, ve, mu2, OP.subtract)
                rstd = rot.tile([P, 1], f32, tag="rstd", bufs=4, name=f"rstd2_{t}")
                nc.vector.reciprocal_approx_fast(out=rstd, in_=ve)
                nc.scalar.activation(rstd, rstd, AF.Sqrt, scale=1.0)
                nmr = rot.tile([P, 1], f32, tag="nmr", bufs=4, name=f"nmr2_{t}")
                nc.vector.tensor_tensor(nmr, mu, rstd, OP.mult)
                nc.vector.tensor_scalar_mul(out=nmr, in0=nmr, scalar1=-1.0)
                xn = rot.tile([P, D], bf16, tag="xn", bufs=4, name=f"xn2_{t}")
                nc.vector.tensor_scalar(
                    out=xn, in0=x2[:, t], scalar1=rstd, scalar2=nmr,
                    op0=OP.mult, op1=OP.add,
                )
                pt = ps_c.tile([P, KC, P], bf16, tag="tp", bufs=3, name=f"pt2_{t}")
                for f in range(KC):
                    nc.tensor.transpose(pt[:, f], xn[:, f * P : (f + 1) * P], ident)
                nc.vector.tensor_copy(out=h2T[:, :, t * P : (t + 1) * P], in_=pt)
                # after LN2 consumed x2[t], fold the final-residual b2 in-place
                nc.vector.tensor_tensor(x2[:, t], x2[:, t], b2_b, OP.add)

        # ================= Phase 6: FFN =================
        with nc.named_scope("ffn"):
            w1_sb = wpool.tile([P, KC, D], bf16, tag="w", bufs=2, name="w1_sb")
            nc.sync.dma_start(out=w1_sb, in_=w1_d[:].rearrange("(c p) n -> p c n", p=P))
            for m in range(KC):
                for n in range(NQ // 512):
                    ps = ps_c.tile([P, 512], f32, tag="mm", bufs=4, name=f"psf{m}_{n}")
                    for kc in range(KC):
                        nc.tensor.matmul(
                            ps,
                            lhsT=w1_sb[:, kc, m * P : (m + 1) * P],
                            rhs=h2T[:, kc, n * 512 : (n + 1) * 512],
                            start=(kc == 0), stop=(kc == KC - 1),
                        )
                    nc.scalar.activation(
                        gT[:, m, n * 512 : (n + 1) * 512], ps,
                        AF.Gelu if gelu else AF.Tanh,
                        bias=b1_sb[:, m : m + 1], scale=1.0,
                    )
            w2_sb = wpool.tile([P, KC, D], bf16, tag="w", bufs=2, name="w2_sb")
            nc.sync.dma_start(out=w2_sb, in_=w2_d[:].rearrange("(c p) n -> p c n", p=P))
            out_r = out_d[:].rearrange("(c p) d -> c p d", p=P)
            for qm in range(QCH):
                osb = rot.tile([P, D], f32, tag="osb", bufs=2, name=f"osb{qm}")
                for n2 in range(2):
                    ns = slice(n2 * 384, (n2 + 1) * 384)
                    ps = ps_c.tile([P, 384], f32, tag="mm", bufs=4, name=f"psg{qm}_{n2}")
                    for kc in range(KC):
                        nc.tensor.matmul(
                            ps,
                            lhsT=gT[:, kc, qm * P : (qm + 1) * P],
                            rhs=w2_sb[:, kc, ns],
                            start=(kc == 0), stop=(kc == KC - 1),
                        )
                    nc.vector.tensor_tensor(osb[:, ns], ps, x2[:, qm, ns], OP.add)
                nc.sync.dma_start(out=out_r[qm], in_=osb)
        lpool.release()
        ps_c.release()

    nc.finalize()
    return nc


def _prepare_in_maps(inputs):
    x = np.ascontiguousarray(np.asarray(inputs["x"], dtype=np.float32))
    mask = np.asarray(inputs["attention_mask"])
    ln1_g = np.asarray(inputs["ln1_g"], dtype=np.float64)
    ln1_b = np.asarray(inputs["ln1_b"], dtype=np.float64)
    ln2_g = np.asarray(inputs["ln2_g"], dtype=np.float64)
    ln2_b = np.asarray(inputs["ln2_b"], dtype=np.float64)
    Wq = np.asarray(inputs["Wq"], dtype=np.float64)
    Wk = np.asarray(inputs["Wk"], dtype=np.float64)
    Wv = np.asarray(inputs["Wv"], dtype=np.float64)
    Wo = np.asarray(inputs["Wo"], dtype=np.float64)
    W1 = np.asarray(inputs["W1"], dtype=np.float64)
    W2 = np.asarray(inputs["W2"], dtype=np.float64)
    bo = np.asarray(inputs["bo"], dtype=np.float64)
    b1 = np.asarray(inputs["b1"], dtype=np.float64)
    b2 = np.asarray(inputs["b2"], dtype=np.float64)

    # fold LN gains/biases into the projection weights; x32 pre-scale keeps
    # the 0.02-std weights out of fp8e4's subnormal range (descaled at evac)
    wq_f = (ln1_g[:, None] * Wq * WS).astype(FP8)
    wk_f = (ln1_g[:, None] * Wk * WS).astype(FP8)
    wv_f = (ln1_g[:, None] * Wv * WS).astype(FP8)
    bq = (ln1_b @ Wq).astype(np.float32)
    bk = (ln1_b @ Wk).astype(np.float32)
    bv = ln1_b @ Wv
    wo_f = (Wo * WS).astype(FP8)
    bo2 = (bo + bv @ Wo).astype(np.float32)  # V-bias adds uniformly post-softmax
    w1_f = (ln2_g[:, None] * W1).astype(BF16)
    b1f = (b1 + ln2_b @ W1).astype(np.float32)
    w2_f = W2.astype(BF16)
    b2f = b2.astype(np.float32)

    mask01 = (mask != 0).astype(np.float32)           # [B, S]
    maskv = (mask01 * (1.0 / WS)).astype(np.float32)  # folded into V evac
    maskrep = np.zeros((B, S, H, VW), dtype=FP8)  # [B, S, H, VW] col HS=mask
    maskrep[:, :, :, HS] = mask01.astype(FP8)[:, :, None]

    in_maps = []
    for c in range(8):
        b, half = divmod(c, 2)
        xb = np.roll(x[b], -half * NQ, axis=0)
        mvb = np.roll(maskv[b], -half * NQ, axis=0)
        # [NT, H, VW] -> [P, TCH, H, VW] with token t = c*P + p
        mrb = np.roll(maskrep[b], -half * NQ, axis=0)
        mrb = mrb.reshape(TCH, P, H, VW).transpose(1, 0, 2, 3)
        in_maps.append(
            {
                "x_local": np.ascontiguousarray(xb),
                "maskv": np.ascontiguousarray(mvb),
                "maskrep": np.ascontiguousarray(mrb),
                "wq": wq_f, "wk": wk_f, "wv": wv_f, "wo": wo_f,
                "w1": w1_f, "w2": w2_f,
                "bq": bq, "bk": bk, "bo2": bo2, "b1f": b1f, "b2f": b2f,
            }
        )
    return in_maps


def run_on_cores(inputs, **spmd_kwargs):
    """Build (cached), run on cores 0-7, return (full_output, BassKernelResults)."""
    from concourse.bass_utils import run_bass_kernel_spmd

    if "nc" not in _PROGRAM_CACHE:
        _PROGRAM_CACHE["nc"] = _build_program()
    nc = _PROGRAM_CACHE["nc"]
    in_maps = _prepare_in_maps(inputs)
    res = run_bass_kernel_spmd(nc, in_maps, core_ids=list(range(8)), **spmd_kwargs)
    out = np.empty((B, S, D), dtype=np.float32)
    for c in range(8):
        b, half = divmod(c, 2)
        out[b, half * NQ : (half + 1) * NQ] = res.results[c]["out"]
    return out, res


def kernel(**inputs):
    out, _ = run_on_cores(inputs)
    return out


# revision 19
# speedup vs baseline: 1.0330x; 1.0330x over previous
"""Trainium2 Bass kernel for nn_Block_2010044694563 (dense transformer block).

B=4, S=2048, D=768, H=12 heads of 64. 8 NeuronCores, no collectives:
core c handles batch c//2, query-half c%2. Each core receives its batch's
2048 tokens rolled so its 1024 query rows come first, computes LN1 + K/V
over all 2048 local tokens (the only redundant work), attention for its
1024 queries x 12 heads, then out-proj + FFN on its 1024 rows.

Precision: fp32 storage / elementwise / PSUM accumulation. QKV/out-proj
matmuls run in fp8e4 DoubleRow perf mode (2 contraction rows per
partition, half-rate rows): weights are pre-scaled x32 on the host so
their 0.02-std values clear the fp8 subnormal cliff, and the 1/32 (and
attention's 1/64 fp8 range scale) are folded into the PSUM-evacuation
scalar ops. Scores stay bf16 (64-deep contraction can't use DoubleRow);
softmax exp emits fp8 directly from the ACT engine and PV consumes it
with V stationary in DoubleRow over key-chunk pairs. The attention mask
is folded into V and the denominator ones-column (zeroed keys drop out
of both numerator and denominator), so exp needs no per-key bias and
LN stats run on the vector engine to keep ACT free for exp.
FFN stays bf16 for the error budget.
"""

import numpy as np
import ml_dtypes

B, S, D, H = 4, 2048, 768, 12
HS = D // H           # 64
P = 128
NT = S                # local tokens per core (whole batch)
NQ = S // 2           # query tokens per core
TCH = NT // P         # 16 token chunks
QCH = NQ // P         # 8 query chunks
KC = D // P           # 6 feature chunks
EPS = 1e-5
SCALE = float(D) ** -0.5
VW = 96            # V columns padded to a 32-multiple for DoubleRow ldweights
BF16 = ml_dtypes.bfloat16
FP8 = ml_dtypes.float8_e4m3
WS = 32.0             # host-side fp8 weight scale
AS = 64.0             # attention-probs fp8 range scale

_PROGRAM_CACHE = {}


def _build_program(gelu=True):
    import concourse.bass as bass
    import concourse.mybir as mybir
    import concourse.tile as tile
    from concourse import bacc
    from concourse.masks import make_identity
    from contextlib import ExitStack

    f32 = mybir.dt.float32
    bf16 = mybir.dt.bfloat16
    fp8 = mybir.dt.float8e4
    AF = mybir.ActivationFunctionType
    OP = mybir.AluOpType
    DR = mybir.MatmulPerfMode.DoubleRow

    nc = bacc.Bacc(None, target_bir_lowering=False)

    x_d = nc.dram_tensor("x_local", [NT, D], f32, kind="ExternalInput")
    mv_d = nc.dram_tensor("maskv", [NT], f32, kind="ExternalInput")
    mr_d = nc.dram_tensor("maskrep", [P, TCH, H, VW], fp8, kind="ExternalInput")
    wq_d = nc.dram_tensor("wq", [D, D], fp8, kind="ExternalInput")
    wk_d = nc.dram_tensor("wk", [D, D], fp8, kind="ExternalInput")
    wv_d = nc.dram_tensor("wv", [D, D], fp8, kind="ExternalInput")
    wo_d = nc.dram_tensor("wo", [D, D], fp8, kind="ExternalInput")
    w1_d = nc.dram_tensor("w1", [D, D], bf16, kind="ExternalInput")
    w2_d = nc.dram_tensor("w2", [D, D], bf16, kind="ExternalInput")
    bq_d = nc.dram_tensor("bq", [D], f32, kind="ExternalInput")
    bk_d = nc.dram_tensor("bk", [D], f32, kind="ExternalInput")
    bo_d = nc.dram_tensor("bo2", [D], f32, kind="ExternalInput")
    b1_d = nc.dram_tensor("b1f", [D], f32, kind="ExternalInput")
    b2_d = nc.dram_tensor("b2f", [D], f32, kind="ExternalInput")
    out_d = nc.dram_tensor("out", [NQ, D], f32, kind="ExternalOutput")

    with tile.TileContext(nc) as tc, ExitStack() as ctx:
        const = ctx.enter_context(tc.tile_pool(name="const", bufs=1))
        glob = ctx.enter_context(tc.tile_pool(name="glob", bufs=1))
        rot = ctx.enter_context(tc.tile_pool(name="rot", bufs=1))
        wpool = ctx.enter_context(tc.tile_pool(name="wpool", bufs=1))

        # ---- constants ----
        ident = const.tile([P, P], bf16)
        make_identity(nc, ident)
        mv_sb = const.tile([P, TCH], f32)
        nc.sync.dma_start(out=mv_sb, in_=mv_d[:].rearrange("(c p) -> p c", p=P))
        bq_sb = const.tile([P, KC], f32)
        nc.sync.dma_start(out=bq_sb, in_=bq_d[:].rearrange("(c p) -> p c", p=P))
        bk_sb = const.tile([P, KC], f32)
        nc.sync.dma_start(out=bk_sb, in_=bk_d[:].rearrange("(c p) -> p c", p=P))
        b1_sb = const.tile([P, KC], f32)
        nc.sync.dma_start(out=b1_sb, in_=b1_d[:].rearrange("(c p) -> p c", p=P))
        # per-feature biases broadcast across partitions (token-major use)
        bo_b = const.tile([P, D], f32)
        _bo = bo_d[:]
        nc.gpsimd.dma_start(
            out=bo_b, in_=bass.AP(tensor=_bo.tensor, offset=_bo.offset, ap=[[0, P], _bo.ap[0]])
        )
        b2_b = const.tile([P, D], f32)
        _b2 = b2_d[:]
        nc.gpsimd.dma_start(
            out=b2_b, in_=bass.AP(tensor=_b2.tensor, offset=_b2.offset, ap=[[0, P], _b2.ap[0]])
        )

        # whole-kernel persistent: attention output (feature-major, normalized)
        oT = glob.tile([P, KC, NQ], fp8)
        xq = glob.tile([P, QCH, D], f32)

        x_r = x_d[:].rearrange("(c p) d -> c p d", p=P)

        # phase-scoped pools (stack order: apool outlives hpool)
        apool = tc.alloc_tile_pool(name="apool", bufs=1)
        hpool = tc.alloc_tile_pool(name="hpool", bufs=1)
        ps_a = tc.alloc_tile_pool(name="ps_a", bufs=1, space="PSUM")

        qT = apool.tile([P, KC, NQ], bf16)       # Q^T (head-pair-major)
        kT = apool.tile([P, KC, NT], bf16)       # K^T
        vA = apool.tile([P, TCH, H, VW], fp8)  # V/32 per (tok chunk, head): [V | mask | 0pad]
        hT = hpool.tile([P, KC, NT], fp8)        # LN1(x)^T, feature-major

        # full-width image of vA: zeros where V lands (overwritten by the
        # projection evacs), mask in the denominator column, zero pad to the
        # 32-multiple stationary width DoubleRow needs. One contiguous DMA.
        nc.sync.dma_start(out=vA[:, :, :, :], in_=mr_d[:, :, :, :])

        # ================= Phase 1: LN1 + transpose to h^T =================
        with nc.named_scope("ln1"):
            for t in range(TCH):
                xt = rot.tile([P, D], f32, tag="xin", bufs=3, name=f"xt{t}")
                nc.sync.dma_start(out=xt, in_=x_r[t])
                scr = rot.tile([P, D], bf16, tag="xn", bufs=4, name=f"scr{t}")
                ssq = rot.tile([P, 1], f32, tag="ssq", bufs=4, name=f"ssq{t}")
                msum = rot.tile([P, 1], f32, tag="msum", bufs=4, name=f"msum{t}")
                nc.scalar.activation(scr, xt, AF.Square, accum_out=ssq)
                nc.vector.reduce_sum(out=msum, in_=xt, axis=mybir.AxisListType.X)
                # var = ssq/D - (msum/D)^2 ; rstd = sqrt(1/(var+eps))
                mu = rot.tile([P, 1], f32, tag="mu", bufs=4, name=f"mu{t}")
                nc.vector.tensor_scalar_mul(out=mu, in0=msum, scalar1=1.0 / D)
                mu2 = rot.tile([P, 1], f32, tag="mu2", bufs=4, name=f"mu2{t}")
                nc.vector.tensor_tensor(mu2, mu, mu, OP.mult)
                ve = rot.tile([P, 1], f32, tag="ve", bufs=4, name=f"ve_{t}")
                nc.vector.tensor_scalar(
                    out=ve, in0=ssq, scalar1=1.0 / D, scalar2=EPS,
                    op0=OP.mult, op1=OP.add,
                )
                nc.vector.tensor_tensor(ve, ve, mu2, OP.subtract)
                rstd = rot.tile([P, 1], f32, tag="rstd", bufs=4, name=f"rstd{t}")
                nc.vector.reciprocal_approx_fast(out=rstd, in_=ve)
                nc.scalar.activation(rstd, rstd, AF.Sqrt, scale=1.0)
                nmr = rot.tile([P, 1], f32, tag="nmr", bufs=4, name=f"nmr{t}")
                nc.vector.tensor_tensor(nmr, mu, rstd, OP.mult)
                nc.vector.tensor_scalar_mul(out=nmr, in0=nmr, scalar1=-1.0)
                xn = rot.tile([P, D], bf16, tag="xn", bufs=4, name=f"xn{t}")
                nc.vector.tensor_scalar(
                    out=xn, in0=xt, scalar1=rstd, scalar2=nmr,
                    op0=OP.mult, op1=OP.add,
                )
                pt = ps_a.tile([P, KC, P], bf16, tag="tp", bufs=3, name=f"pt{t}")
                for f in range(KC):
                    nc.tensor.transpose(pt[:, f], xn[:, f * P : (f + 1) * P], ident)
                nc.vector.tensor_copy(out=hT[:, :, t * P : (t + 1) * P], in_=pt)

        # residual rows for the out-projection: queued after LN1's x loads
        for t in range(QCH):
            nc.sync.dma_start(out=xq[:, t], in_=x_r[t])
            nc.vector.tensor_tensor(xq[:, t], xq[:, t], bo_b, OP.add)

        # ================= Phase 2: Q/K/V projections (fp8 DoubleRow) ======
        with nc.named_scope("qkv"):
            wv_sb = wpool.tile([P, KC, D], fp8, tag="w8", bufs=3, name="wv_sb")
            nc.sync.dma_start(out=wv_sb, in_=wv_d[:].rearrange("(c p) n -> p c n", p=P))
            wq_sb = wpool.tile([P, KC, D], fp8, tag="w8", bufs=3, name="wq_sb")
            nc.sync.dma_start(out=wq_sb, in_=wq_d[:].rearrange("(c p) n -> p c n", p=P))
            wk_sb = wpool.tile([P, KC, D], fp8, tag="w8", bufs=3, name="wk_sb")
            nc.sync.dma_start(out=wk_sb, in_=wk_d[:].rearrange("(c p) n -> p c n", p=P))
            for t in range(TCH):
                for n2 in range(2):
                    ps = ps_a.tile([P, 384], f32, tag="mm", bufs=4, name=f"psv{t}_{n2}")
                    for c in range(KC // 2):
                        nc.tensor.matmul(
                            ps,
                            lhsT=hT[:, 2 * c : 2 * c + 2, t * P : (t + 1) * P],
                            rhs=wv_sb[:, 2 * c : 2 * c + 2, n2 * 384 : (n2 + 1) * 384],
                            start=(c == 0), stop=(c == KC // 2 - 1),
                            perf_mode=DR,
                        )
                    # x(1/32) de-scales the fp8 weights; mask zeroes dead keys
                    nc.vector.tensor_scalar(
                        out=vA[:, t, n2 * 6 : (n2 + 1) * 6, 0:HS],
                        in0=ps.rearrange("p (h d) -> p h d", h=6),
                        scalar1=mv_sb[:, t : t + 1],
                        scalar2=None,
                        op0=OP.mult,
                    )
        ps_a.release()

        # ================= Phase 3: attention ==============================
        # scores^T[k,q] per head (bf16, contraction=64), exp fused with the
        # D^-0.5 scale straight to fp8, PV with V stationary in DoubleRow over
        # key-chunk pairs; the mask column of V gives denominators for free.
        wo_sb = wpool.tile([P, KC, D], fp8, tag="w8", bufs=3, name="wo_sb")
        nc.sync.dma_start(out=wo_sb, in_=wo_d[:].rearrange("(c p) n -> p c n", p=P))
        ps_b = tc.alloc_tile_pool(name="ps_b", bufs=1, space="PSUM")
        dpool = tc.alloc_tile_pool(name="dpool", bufs=1, space="DRAM")

        def emit_norm(pv, h, qc):
            # deferred softmax-normalize: AS/denom (fast recip via SBUF copy),
            # replicate across partitions through a DRAM-roundtrip broadcast
            # DMA, then scale O rows during the PSUM evacuation.
            qs = slice(qc * 512, (qc + 1) * 512)
            hr = slice((h % 2) * HS, (h % 2) * HS + HS)
            pvr = rot.tile([1, 512], f32, tag="pvr", bufs=2, name=f"pvr{h}_{qc}")
            nc.vector.tensor_scalar_mul(out=pvr, in0=pv[HS : HS + 1, :], scalar1=1.0 / AS)
            rsb = rot.tile([1, 512], f32, tag="rsb", bufs=2, name=f"rsb{h}_{qc}")
            nc.vector.reciprocal_approx_fast(out=rsb, in_=pvr)
            rd = dpool.tile([1, 512], f32, tag="rd", bufs=2, name=f"rd{h}_{qc}")
            nc.sync.dma_start(out=rd, in_=rsb)
            rrs = rot.tile([HS, 512], f32, tag="rrs", bufs=2, name=f"rrs{h}_{qc}")
            nc.gpsimd.dma_start(
                out=rrs,
                in_=bass.AP(
                    tensor=rd.tensor, offset=rd.offset,
                    ap=[[0, HS]] + [list(a) for a in rd.ap[1:]],
                ),
            )
            nc.vector.tensor_tensor(oT[hr, h // 2, qs], pv[0:HS, :], rrs, OP.mult)

        def qk_proj(hp):
            # fp8 DoubleRow Q/K projection for head-pair hp, emitted inside
            # the attention loop so the PE stays fed during ACT-bound spans
            for n in range(NQ // 512):
                psq = ps_b.tile([P, 512], f32, tag="qk", bufs=2, name=f"psq{hp}_{n}")
                for c in range(KC // 2):
                    nc.tensor.matmul(
                        psq,
                        lhsT=wq_sb[:, 2 * c : 2 * c + 2, hp * P : (hp + 1) * P],
                        rhs=hT[:, 2 * c : 2 * c + 2, n * 512 : (n + 1) * 512],
                        start=(c == 0), stop=(c == KC // 2 - 1),
                        perf_mode=DR,
                    )
                nc.vector.tensor_scalar(
                    out=qT[:, hp, n * 512 : (n + 1) * 512], in0=psq,
                    scalar1=1.0 / WS, scalar2=bq_sb[:, hp : hp + 1],
                    op0=OP.mult, op1=OP.add,
                )
            for n in range(NT // 512):
                psk = ps_b.tile([P, 512], f32, tag="qk", bufs=2, name=f"psk{hp}_{n}")
                for c in range(KC // 2):
                    nc.tensor.matmul(
                        psk,
                        lhsT=wk_sb[:, 2 * c : 2 * c + 2, hp * P : (hp + 1) * P],
                        rhs=hT[:, 2 * c : 2 * c + 2, n * 512 : (n + 1) * 512],
                        start=(c == 0), stop=(c == KC // 2 - 1),
                        perf_mode=DR,
                    )
                nc.vector.tensor_scalar(
                    out=kT[:, hp, n * 512 : (n + 1) * 512], in0=psk,
                    scalar1=1.0 / WS, scalar2=bk_sb[:, hp : hp + 1],
                    op0=OP.mult, op1=OP.add,
                )

        with nc.named_scope("attn"):
            pending = None
            for h in range(H):
                if h % 2 == 0:
                    qk_proj(h // 2)
                hd = slice((h % 2) * HS, (h % 2) * HS + HS)
                for qc in range(NQ // 512):
                    qs = slice(qc * 512, (qc + 1) * 512)
                    pv = ps_b.tile([VW, 512], f32, tag="pv", bufs=2, name=f"pv{h}_{qc}")
                    for jp in range(TCH // 2):
                        sc = ps_b.tile([P, 2, 512], f32, tag="sc", bufs=2, name=f"sc{h}_{qc}_{jp}")
                        for jj in range(2):
                            js = slice((2 * jp + jj) * P, (2 * jp + jj + 1) * P)
                            nc.tensor.matmul(
                                sc[:, jj, :], lhsT=kT[hd, h // 2, js], rhs=qT[hd, h // 2, qs],
                                start=True, stop=True,
                            )
                        ex = rot.tile([P, 2, 512], fp8, tag="expT", bufs=3, name=f"ex{h}_{qc}_{jp}")
                        nc.scalar.activation(ex, sc, AF.Exp, scale=SCALE)
                        nc.tensor.matmul(
                            pv,
                            lhsT=vA[:, 2 * jp : 2 * jp + 2, h, :],
                            rhs=ex,
                            start=(jp == 0), stop=(jp == TCH // 2 - 1),
                            perf_mode=DR,
                        )
                    if pending is not None:
                        emit_norm(*pending)
                    pending = (pv, h, qc)
            emit_norm(*pending)
        hpool.release()
        apool.release()
        ps_b.release()
        dpool.release()

        # ================= Phase 4: out-projection + residual ==============
        lpool = tc.alloc_tile_pool(name="lpool", bufs=1)
        ps_c = tc.alloc_tile_pool(name="ps_c", bufs=1, space="PSUM")
        x2 = lpool.tile([P, QCH, D], f32)
        h2T = lpool.tile([P, KC, NQ], bf16)
        gT = lpool.tile([P, KC, NQ], bf16)
        with nc.named_scope("proj"):
            for qm in range(QCH):
                for n2 in range(2):
                    ns = slice(n2 * 384, (n2 + 1) * 384)
                    ps = ps_c.tile([P, 384], f32, tag="mm", bufs=4, name=f"pso{qm}_{n2}")
                    for c in range(KC // 2):
                        nc.tensor.matmul(
                            ps,
                            lhsT=oT[:, 2 * c : 2 * c + 2, qm * P : (qm + 1) * P],
                            rhs=wo_sb[:, 2 * c : 2 * c + 2, ns],
                            start=(c == 0), stop=(c == KC // 2 - 1),
                            perf_mode=DR,
                        )
                    # 1/(AS*WS) undoes the attn fp8 range and weight scales
                    osc = rot.tile([P, 384], bf16, tag="osc", bufs=3, name=f"osc{qm}_{n2}")
                    nc.vector.tensor_scalar_mul(out=osc, in0=ps, scalar1=1.0 / (AS * WS))
                    nc.vector.tensor_tensor(x2[:, qm, ns], osc, xq[:, qm, ns], OP.add)

        # ================= Phase 5: LN2 + transpose =================
        with nc.named_scope("ln2"):
            for t in range(QCH):
                scr = rot.tile([P, D], bf16, tag="xn", bufs=4, name=f"scr2_{t}")
                ssq = rot.tile([P, 1], f32, tag="ssq", bufs=4, name=f"ssq2_{t}")
                msum = rot.tile([P, 1], f32, tag="msum", bufs=4, name=f"msum2_{t}")
                nc.scalar.activation(scr, x2[:, t], AF.Square, accum_out=ssq)
                nc.vector.reduce_sum(out=msum, in_=x2[:, t], axis=mybir.AxisListType.X)
                mu = rot.tile([P, 1], f32, tag="mu", bufs=4, name=f"mu_2{t}")
                nc.vector.tensor_scalar_mul(out=mu, in0=msum, scalar1=1.0 / D)
                mu2 = rot.tile([P, 1], f32, tag="mu2", bufs=4, name=f"mu2_2{t}")
                nc.vector.tensor_tensor(mu2, mu, mu, OP.mult)
                ve = rot.tile([P, 1], f32, tag="ve", bufs=4, name=f"ve2_{t}")
                nc.vector.tensor_scalar(
                    out=ve, in0=ssq, scalar1=1.0 / D, scalar2=EPS,
                    op0=OP.mult, op1=OP.add,
                )
                nc.vector.tensor_tensor(ve, ve, mu2, OP.subtract)
                rstd = rot.tile([P, 1], f32, tag="rstd", bufs=4, name=f"rstd2_{t}")
                nc.vector.reciprocal_approx_fast(out=rstd, in_=ve)
                nc.scalar.activation(rstd, rstd, AF.Sqrt, scale=1.0)
                nmr = rot.tile([P, 1], f32, tag="nmr", bufs=4, name=f"nmr2_{t}")
                nc.vector.tensor_tensor(nmr, mu, rstd, OP.mult)
                nc.vector.tensor_scalar_mul(out=nmr, in0=nmr, scalar1=-1.0)
                xn = rot.tile([P, D], bf16, tag="xn", bufs=4, name=f"xn2_{t}")
                nc.vector.tensor_scalar(
                    out=xn, in0=x2[:, t], scalar1=rstd, scalar2=nmr,
                    op0=OP.mult, op1=OP.add,
                )
                pt = ps_c.tile([P, KC, P], bf16, tag="tp", bufs=3, name=f"pt2_{t}")
                for f in range(KC):
                    nc.tensor.transpose(pt[:, f], xn[:, f * P : (f + 1) * P], ident)
                nc.vector.tensor_copy(out=h2T[:, :, t * P : (t + 1) * P], in_=pt)
                # after LN2 consumed x2[t], fold the final-residual b2 in-place
                nc.vector.tensor_tensor(x2[:, t], x2[:, t], b2_b, OP.add)

        # ================= Phase 6: FFN =================
        with nc.named_scope("ffn"):
            w1_sb = wpool.tile([P, KC, D], bf16, tag="w", bufs=2, name="w1_sb")
            nc.sync.dma_start(out=w1_sb, in_=w1_d[:].rearrange("(c p) n -> p c n", p=P))
            for m in range(KC):
                for n in range(NQ // 512):
                    ps = ps_c.tile([P, 512], f32, tag="mm", bufs=4, name=f"psf{m}_{n}")
                    for kc in range(KC):
                        nc.tensor.matmul(
                            ps,
                            lhsT=w1_sb[:, kc, m * P : (m + 1) * P],
                            rhs=h2T[:, kc, n * 512 : (n + 1) * 512],
                            start=(kc == 0), stop=(kc == KC - 1),
                        )
                    nc.scalar.activation(
                        gT[:, m, n * 512 : (n + 1) * 512], ps,
                        AF.Gelu if gelu else AF.Tanh,
                        bias=b1_sb[:, m : m + 1], scale=1.0,
                    )
            w2_sb = wpool.tile([P, KC, D], bf16, tag="w", bufs=2, name="w2_sb")
            nc.sync.dma_start(out=w2_sb, in_=w2_d[:].rearrange("(c p) n -> p c n", p=P))
            out_r = out_d[:].rearrange("(c p) d -> c p d", p=P)
            for qm in range(QCH):
                osb = rot.tile([P, D], f32, tag="osb", bufs=2, name=f"osb{qm}")
                for n2 in range(2):
                    ns = slice(n2 * 384, (n2 + 1) * 384)
                    ps = ps_c.tile([P, 384], f32, tag="mm", bufs=4, name=f"psg{qm}_{n2}")
                    for kc in range(KC):
                        nc.tensor.matmul(
                            ps,
                            lhsT=gT[:, kc, qm * P : (qm + 1) * P],
                            rhs=w2_sb[:, kc, ns],
                            start=(kc == 0), stop=(kc == KC - 1),
                        )
                    nc.vector.tensor_tensor(osb[:, ns], ps, x2[:, qm, ns], OP.add)
                nc.sync.dma_start(out=out_r[qm], in_=osb)
        lpool.release()
        ps_c.release()

    nc.finalize()
    return nc


def _prepare_in_maps(inputs):
    x = np.ascontiguousarray(np.asarray(inputs["x"], dtype=np.float32))
    mask = np.asarray(inputs["attention_mask"])
    ln1_g = np.asarray(inputs["ln1_g"], dtype=np.float64)
    ln1_b = np.asarray(inputs["ln1_b"], dtype=np.float64)
    ln2_g = np.asarray(inputs["ln2_g"], dtype=np.float64)
    ln2_b = np.asarray(inputs["ln2_b"], dtype=np.float64)
    Wq = np.asarray(inputs["Wq"], dtype=np.float64)
    Wk = np.asarray(inputs["Wk"], dtype=np.float64)
    Wv = np.asarray(inputs["Wv"], dtype=np.float64)
    Wo = np.asarray(inputs["Wo"], dtype=np.float64)
    W1 = np.asarray(inputs["W1"], dtype=np.float64)
    W2 = np.asarray(inputs["W2"], dtype=np.float64)
    bo = np.asarray(inputs["bo"], dtype=np.float64)
    b1 = np.asarray(inputs["b1"], dtype=np.float64)
    b2 = np.asarray(inputs["b2"], dtype=np.float64)

    # fold LN gains/biases into the projection weights; x32 pre-scale keeps
    # the 0.02-std weights out of fp8e4's subnormal range (descaled at evac)
    wq_f = (ln1_g[:, None] * Wq * WS).astype(FP8)
    wk_f = (ln1_g[:, None] * Wk * WS).astype(FP8)
    wv_f = (ln1_g[:, None] * Wv * WS).astype(FP8)
    bq = (ln1_b @ Wq).astype(np.float32)
    bk = (ln1_b @ Wk).astype(np.float32)
    bv = ln1_b @ Wv
    wo_f = (Wo * WS).astype(FP8)
    bo2 = (bo + bv @ Wo).astype(np.float32)  # V-bias adds uniformly post-softmax
    w1_f = (ln2_g[:, None] * W1).astype(BF16)
    b1f = (b1 + ln2_b @ W1).astype(np.float32)
    w2_f = W2.astype(BF16)
    b2f = b2.astype(np.float32)

    mask01 = (mask != 0).astype(np.float32)           # [B, S]
    maskv = (mask01 * (1.0 / WS)).astype(np.float32)  # folded into V evac
    maskrep = np.zeros((B, S, H, VW), dtype=FP8)  # [B, S, H, VW] col HS=mask
    maskrep[:, :, :, HS] = mask01.astype(FP8)[:, :, None]

    in_maps = []
    for c in range(8):
        b, half = divmod(c, 2)
        xb = np.roll(x[b], -half * NQ, axis=0)
        mvb = np.roll(maskv[b], -half * NQ, axis=0)
        # [NT, H, VW] -> [P, TCH, H, VW] with token t = c*P + p
        mrb = np.roll(maskrep[b], -half * NQ, axis=0)
        mrb = mrb.reshape(TCH, P, H, VW).transpose(1, 0, 2, 3)
        in_maps.append(
            {
                "x_local": np.ascontiguousarray(xb),
                "maskv": np.ascontiguousarray(mvb),
                "maskrep": np.ascontiguousarray(mrb),
                "wq": wq_f, "wk": wk_f, "wv": wv_f, "wo": wo_f,
                "w1": w1_f, "w2": w2_f,
                "bq": bq, "bk": bk, "bo2": bo2, "b1f": b1f, "b2f": b2f,
            }
        )
    return in_maps


def run_on_cores(inputs, **spmd_kwargs):
    """Build (cached), run on cores 0-7, return (full_output, BassKernelResults)."""
    from concourse.bass_utils import run_bass_kernel_spmd

    if "nc" not in _PROGRAM_CACHE:
        _PROGRAM_CACHE["nc"] = _build_program()
    nc = _PROGRAM_CACHE["nc"]
    in_maps = _prepare_in_maps(inputs)
    res = run_bass_kernel_spmd(nc, in_maps, core_ids=list(range(8)), **spmd_kwargs)
    out = np.empty((B, S, D), dtype=np.float32)
    for c in range(8):
        b, half = divmod(c, 2)
        out[b, half * NQ : (half + 1) * NQ] = res.results[c]["out"]
    return out, res


def kernel(**inputs):
    out, _ = run_on_cores(inputs)
    return out
